# revision 4
# baseline (speedup 1.0000x reference)
"""EyeLoss Trainium2 kernel (nn_EyeLoss_83691732730572).

Key observation: the reference only ever consumes channels 96 and 97 of the
98-channel heatmaps (eyes = landmarks[:, 96/97], MSE over [:, 96/97]), so of
the 2 x 128 x 98 x 64 x 64 f32 input only 8 MB is live. The kernel shards
batches across 8 NeuronCores (16 per core), ships only the two live channels,
and per (batch, channel, tensor) map computes on-device:

  - argmax over the 64x64 map (DVE Max8 + MaxIndex, map split into 2
    half-maps of 2048 elems so all 128 partitions are busy)
  - the x/y neighbor differences at the argmax via an eq-mask fused
    multiply-accumulate (scalar_tensor_tensor is_equal/mult with accum_out)
    against shifted-difference maps (gpsimd), using 1-row halos so the
    +-64 shifts never leave the partition
  - the per-map sum of (src - tgt)^2 (ACT Square with accum_out) for the
    two scalar MSE means

Each core returns one packed [128, 8] f32 tile; the host finishes the ~100
flops of subpixel decode per map and the scalar mean reduction.
"""
import numpy as np

import concourse.bass as bass
import concourse.mybir as mybir
from concourse.bass_utils import run_bass_kernel_spmd

F32 = mybir.dt.float32
U32 = mybir.dt.uint32

B, L, H, W = 128, 98, 64, 64
HW = H * W                      # 4096
N_CORES = 8
BPC = B // N_CORES              # 16 batches per core
NMAP = BPC * 2                  # 32 maps per tensor per core (b-major, l minor)
HALF = HW // 2                  # 2048 elems per half-map
HALO = W                        # one image row of halo on each side
ROW = HALF + 2 * HALO           # 2176 cols per partition
PAD_LEN = HALO + NMAP * HW + HALO  # padded flat length per tensor per core

# Optional tracing knobs (used by test.py; harness leaves these alone).
TRACE = False
LAST_RESULTS = None

_CACHED_NC = None


def _build_nc():
    nc = bass.Bass()
    src = nc.dram_tensor("src", [PAD_LEN], F32, kind="ExternalInput")
    tgt = nc.dram_tensor("tgt", [PAD_LEN], F32, kind="ExternalInput")
    outp = nc.dram_tensor("outp", [128, 8], F32, kind="ExternalOutput")

    from contextlib import ExitStack

    with ExitStack() as ctx:
        T = ctx.enter_context(nc.sbuf_tensor("T", [128, ROW], F32))
        U = ctx.enter_context(nc.sbuf_tensor("U", [64, HALF], F32))
        DFX = ctx.enter_context(nc.sbuf_tensor("DFX", [128, HALF], F32))
        DFY = ctx.enter_context(nc.sbuf_tensor("DFY", [128, HALF], F32))
        SCX = ctx.enter_context(nc.sbuf_tensor("SCX", [128, HALF], F32))
        SCY = ctx.enter_context(nc.sbuf_tensor("SCY", [128, HALF], F32))
        DS = ctx.enter_context(nc.sbuf_tensor("DS", [64, HALF], F32))
        SQ = ctx.enter_context(nc.sbuf_tensor("SQ", [64, HALF], F32))
        MAX8 = ctx.enter_context(nc.sbuf_tensor("MAX8", [128, 8], F32))
        IDX8 = ctx.enter_context(nc.sbuf_tensor("IDX8", [128, 8], U32))
        PK = ctx.enter_context(nc.sbuf_tensor("PK", [128, 8], F32))
        ZB = ctx.enter_context(nc.sbuf_tensor("ZB", [128, 1], F32))
        sem_T = ctx.enter_context(nc.semaphore("sem_T"))
        sem_U = ctx.enter_context(nc.semaphore("sem_U"))
        sem_max8 = ctx.enter_context(nc.semaphore("sem_max8"))
        sem_ds = ctx.enter_context(nc.semaphore("sem_ds"))
        sem_dve = ctx.enter_context(nc.semaphore("sem_dve"))
        sem_dfx = ctx.enter_context(nc.semaphore("sem_dfx"))
        sem_dfy = ctx.enter_context(nc.semaphore("sem_dfy"))
        sem_act = ctx.enter_context(nc.semaphore("sem_act"))
        sem_idx = ctx.enter_context(nc.semaphore("sem_idx"))
        dma_out = ctx.enter_context(nc.semaphore("dma_out"))
        block = ctx.enter_context(nc.Block())
        # T partition p = t*64 + h*32 + m  (t: 0=src 1=tgt, h: half, m = 2b+l)
        # T cols: [0,64) left halo | [64, 2112) half-map data | [2112, 2176) right halo
        # padded DRAM: [64 zeros][maps flat][64 zeros]; window for (m, h) starts
        # at padded offset m*4096 + h*2048 and spans 2176 elements.
        @block.sync
        def _(sync):
            for t, dram in ((0, src), (1, tgt)):
                for h in range(2):
                    p0 = t * 64 + h * 32
                    sync.dma_start(
                        T[p0 : p0 + 32, :],
                        bass.AP(dram, h * HALF, [[HW, NMAP], [1, ROW]]),
                    ).then_inc(sem_T, 16)       # -> 64
            # U partition q = h*32 + m mirrors T's src side; data region only.
            for h in range(2):
                sync.dma_start(
                    U[h * 32 : h * 32 + 32, :],
                    bass.AP(tgt, HALO + h * HALF, [[HW, NMAP], [1, HALF]]),
                ).then_inc(sem_U, 16)           # -> 32
            sync.wait_ge(sem_dve, 1)
            sync.wait_ge(sem_act, 1)
            sync.dma_start(outp[:, :], PK[:, :]).then_inc(dma_out, 16)
            sync.wait_ge(dma_out, 16)

        @block.vector
        def _(vector):
            vector.memset(PK[:, :], 0.0)
            vector.memset(ZB[:, :], 0.0)
            vector.wait_ge(sem_T, 64)           # T fully loaded
            vector.max(MAX8[:, :], T[:, HALO : HALO + HALF]).then_inc(sem_max8, 1)
            vector.wait_ge(sem_max8, 1)
            vector.max_index(IDX8[:, :], MAX8[:, :], T[:, HALO : HALO + HALF]).then_inc(sem_idx, 1)
            vector.wait_ge(sem_idx, 1)
            vector.tensor_copy(PK[:, 1:2], IDX8[:, 0:1])
            vector.wait_ge(sem_U, 32)           # U loaded
            vector.tensor_tensor(
                DS[:, :], T[0:64, HALO : HALO + HALF], U[:, :],
                op=mybir.AluOpType.subtract,
            ).then_inc(sem_ds, 1)
            vector.wait_ge(sem_dfx, 1)
            vector.scalar_tensor_tensor(
                SCX[:, :], T[:, HALO : HALO + HALF], MAX8[:, 0:1], DFX[:, :],
                op0=mybir.AluOpType.is_equal, op1=mybir.AluOpType.mult,
                accum_out=PK[:, 2:3],
            )
            vector.wait_ge(sem_dfy, 1)
            vector.scalar_tensor_tensor(
                SCY[:, :], T[:, HALO : HALO + HALF], MAX8[:, 0:1], DFY[:, :],
                op0=mybir.AluOpType.is_equal, op1=mybir.AluOpType.mult,
                accum_out=PK[:, 3:4],
            ).then_inc(sem_dve, 1)

        @block.gpsimd
        def _(gpsimd):
            gpsimd.wait_ge(sem_T, 64)
            # DFX[p, j] = flat[j+1] - flat[j-1];  DFY[p, j] = flat[j+64] - flat[j-64]
            gpsimd.tensor_tensor(
                DFX[:, :], T[:, HALO + 1 : HALO + 1 + HALF],
                T[:, HALO - 1 : HALO - 1 + HALF], op=mybir.AluOpType.subtract,
            ).then_inc(sem_dfx, 1)
            gpsimd.tensor_tensor(
                DFY[:, :], T[:, 2 * HALO : 2 * HALO + HALF],
                T[:, 0:HALF], op=mybir.AluOpType.subtract,
            ).then_inc(sem_dfy, 1)

        @block.scalar
        def _(scalar):
            scalar.wait_ge(sem_max8, 1)
            scalar.copy(PK[:, 0:1], MAX8[:, 0:1])
            scalar.wait_ge(sem_ds, 1)
            scalar.activation(
                SQ[:, :], DS[:, :], mybir.ActivationFunctionType.Square,
                bias=ZB[0:64, 0:1], accum_out=PK[0:64, 4:5],
            ).then_inc(sem_act, 1)

    return nc


def _pad_core(flat32):
    # flat32: [NMAP*HW] f32 contiguous -> [64 zeros | data | 64 zeros]
    out = np.zeros(PAD_LEN, np.float32)
    out[HALO : HALO + NMAP * HW] = flat32
    return out


def kernel(source_heatmap, target_heatmap):
    global _CACHED_NC, LAST_RESULTS
    src = np.asarray(source_heatmap, np.float32)
    tgt = np.asarray(target_heatmap, np.float32)

    # per-core inputs: batches [c*16, (c+1)*16), channels 96..97, flattened
    in_maps = []
    for c in range(N_CORES):
        s = np.ascontiguousarray(src[c * BPC : (c + 1) * BPC, 96:98]).reshape(-1)
        t = np.ascontiguousarray(tgt[c * BPC : (c + 1) * BPC, 96:98]).reshape(-1)
        in_maps.append({"src": _pad_core(s), "tgt": _pad_core(t)})

    if _CACHED_NC is None:
        _CACHED_NC = _build_nc()
    res = run_bass_kernel_spmd(
        _CACHED_NC, in_maps, list(range(N_CORES)), trace=TRACE
    )
    LAST_RESULTS = res

    # ---- host decode of the packed [128, 8] per-core outputs ----
    # partition p = t*64 + h*32 + m;  m = 2*b_local + l  (l: 0 -> ch96, 1 -> ch97)
    pk = np.stack([res.results[c]["outp"] for c in range(N_CORES)])  # [8,128,8]
    pk = pk.astype(np.float64)

    v = pk.reshape(N_CORES, 2, 2, 32, 8)          # [core, t, h, m, col]
    mx = v[..., 0]                                # [core, t, h, m]
    idx = v[..., 1]
    dx = v[..., 2]
    dy = v[..., 3]

    hwin = (mx[:, :, 1, :] > mx[:, :, 0, :]).astype(np.int64)  # [core, t, m]
    sel = np.take_along_axis                       # helper
    idx_w = sel(idx, hwin[:, :, None, :], axis=2)[:, :, 0, :]
    dx_w = sel(dx, hwin[:, :, None, :], axis=2)[:, :, 0, :]
    dy_w = sel(dy, hwin[:, :, None, :], axis=2)[:, :, 0, :]

    flat = hwin * HALF + idx_w.astype(np.int64)    # [core, t, m] in [0, 4096)
    px = (flat % W).astype(np.float32)
    py = (flat // W).astype(np.float32)
    inside = (px > 0) & (px < W - 1) & (py > 0) & (py < H - 1)
    off_x = np.where(inside, np.sign(dx_w).astype(np.float32) * 0.25, 0.0)
    off_y = np.where(inside, np.sign(dy_w).astype(np.float32) * 0.25, 0.0)
    lx = (px + 0.5 + off_x) * 4.0                  # landmark x
    ly = (py + 0.5 + off_y) * 4.0

    # eyes[b] = [x96, y96, x97, y97];  m = 2*b_local + l
    lx = lx.reshape(N_CORES, 2, BPC, 2)            # [core, t, b_local, l]
    ly = ly.reshape(N_CORES, 2, BPC, 2)
    eyes = np.empty((2, B, 4), np.float32)
    for t in range(2):
        exy = np.stack(
            [lx[:, t, :, 0], ly[:, t, :, 0], lx[:, t, :, 1], ly[:, t, :, 1]],
            axis=-1,
        )                                          # [core, b_local, 4]
        eyes[t] = exy.reshape(B, 4)

    # MSE: col 4 on partitions [0, 64) = per (h, m) sum over its half-map
    ms = pk[:, 0:64, 4].reshape(N_CORES, 2, 32)    # [core, h, m]
    ms_l = ms.reshape(N_CORES, 2, BPC, 2)          # [core, h, b_local, l]
    left = ms_l[..., 0].sum() / (B * HW)
    right = ms_l[..., 1].sum() / (B * HW)
    eye_loss = np.float32(left + right)

    return eye_loss, eyes[0], eyes[1]
